# revision 1
# baseline (speedup 1.0000x reference)
"""Trainium2 Bass kernel for softmax(relu(nodevec1 @ nodevec2), axis=1).

nodevec1: [8192, 10] f32, nodevec2: [10, 8192] f32 -> out [8192, 8192] f32.

Strategy (8 NeuronCores, no collectives needed):
- Row-shard nodevec1: core i computes rows [i*1024, (i+1)*1024).
- Host-side prep: split each f32 input into bf16 hi+lo pairs and stack
  along the contraction dim (K=30: h1*h2 + l1*h2 + h1*l2), so the PE runs
  at bf16 speed with ~f32 accuracy. Also pre-transpose the nodevec1 shard
  to the [K, M] layout the PE wants for the stationary operand.
- Per 128-row tile: matmul (K=30) -> PSUM, ACT exp: PSUM -> SBUF f32,
  DVE tensor_scalar max(e,1) with fused row-sum (exp(relu(x)) ==
  max(exp(x),1)), DVE reciprocal, DVE tensor_scalar scale -> bf16 out,
  DMA out. Row softmax is local to each core.
- Output is written bf16 (halves the HBM write) and widened to f32 on the
  host; softmax values are well inside bf16's safe range.
"""

import os

import numpy as np
import ml_dtypes

NODES = 8192
RANK = 10
N_CORES = 8
ROWS_PER_CORE = NODES // N_CORES  # 1024
RT = 128  # rows per tile (SBUF partition dim)
N_RT = ROWS_PER_CORE // RT  # 8
KS = 3 * RANK  # 30: [h1; l1; h1] x [h2; h2; l2]
PSUM_COLS = 2048  # 4 banks per psum tile
MM_N = 512  # one PSUM bank per matmul

_cached_nc = None
LAST_RESULTS = None  # BassKernelResults from the most recent run (for test.py)


def _build():
    import concourse.bass as bass
    import concourse.tile as tile
    from concourse import bacc, mybir

    bf16 = mybir.dt.bfloat16
    f32 = mybir.dt.float32
    AF = mybir.ActivationFunctionType
    OP = mybir.AluOpType

    nc = bacc.Bacc(None, target_bir_lowering=False, debug=False)

    n1s = nc.declare_dram_parameter("n1s", [KS, ROWS_PER_CORE], bf16, isOutput=False)
    n2s = nc.declare_dram_parameter("n2s", [KS, NODES], bf16, isOutput=False)
    out = nc.declare_dram_parameter("out", [ROWS_PER_CORE, NODES], bf16, isOutput=True)

    with tile.TileContext(nc) as tc:
        with (
            tc.tile_pool(name="const", bufs=1) as cpool,
            tc.tile_pool(name="psum", bufs=2, space=bass.MemorySpace.PSUM) as pspool,
            tc.tile_pool(name="e", bufs=2) as epool,
            tc.tile_pool(name="m", bufs=2) as mpool,
            tc.tile_pool(name="o", bufs=2) as opool,
            tc.tile_pool(name="stats", bufs=4) as spool,
        ):
            a1 = cpool.tile([KS, ROWS_PER_CORE], bf16)
            nc.sync.dma_start(a1[:], n1s[:])
            a2 = cpool.tile([KS, NODES], bf16)
            nc.sync.dma_start(a2[:], n2s[:])

            for rt in range(N_RT):
                e = epool.tile([RT, NODES], f32)
                for g in range(NODES // PSUM_COLS):
                    ps = pspool.tile([RT, PSUM_COLS], f32)
                    for c in range(PSUM_COLS // MM_N):
                        col = g * PSUM_COLS + c * MM_N
                        nc.tensor.matmul(
                            ps[:, c * MM_N : (c + 1) * MM_N],
                            a1[:, rt * RT : (rt + 1) * RT],
                            a2[:, col : col + MM_N],
                            start=True,
                            stop=True,
                        )
                    nc.scalar.activation(
                        e[:, g * PSUM_COLS : (g + 1) * PSUM_COLS], ps[:], AF.Exp
                    )

                m = mpool.tile([RT, NODES], bf16)
                z = spool.tile([RT, 1], f32)
                # m = max(e, 1) == exp(relu(scores)); z = row-sum of m
                nc.vector.tensor_scalar(
                    m[:], e[:], 1.0, None, OP.max, OP.add, accum_out=z[:]
                )
                inv = spool.tile([RT, 1], f32)
                nc.vector.reciprocal(inv[:], z[:])
                o = opool.tile([RT, NODES], bf16)
                nc.vector.tensor_scalar(o[:], m[:], inv[:], None, OP.mult, OP.bypass)
                nc.sync.dma_start(out[rt * RT : (rt + 1) * RT, :], o[:])

    nc.compile()
    return nc


def kernel(nodevec1: np.ndarray, nodevec2: np.ndarray) -> np.ndarray:
    from concourse.bass_utils import run_bass_kernel_spmd

    global _cached_nc, LAST_RESULTS
    if _cached_nc is None:
        _cached_nc = _build()
    nc = _cached_nc

    bf = ml_dtypes.bfloat16
    n1 = np.asarray(nodevec1, dtype=np.float32)
    n2 = np.asarray(nodevec2, dtype=np.float32)

    h1 = n1.astype(bf)
    l1 = (n1 - h1.astype(np.float32)).astype(bf)
    h2 = n2.astype(bf)
    l2 = (n2 - h2.astype(np.float32)).astype(bf)

    n2s = np.ascontiguousarray(np.concatenate([h2, h2, l2], axis=0))  # [30, 8192]

    in_maps = []
    for i in range(N_CORES):
        sl = slice(i * ROWS_PER_CORE, (i + 1) * ROWS_PER_CORE)
        n1s_i = np.ascontiguousarray(
            np.concatenate([h1[sl].T, l1[sl].T, h1[sl].T], axis=0)
        )  # [30, 1024]
        in_maps.append({"n1s": n1s_i, "n2s": n2s})

    res = run_bass_kernel_spmd(nc, in_maps, core_ids=list(range(N_CORES)))
    LAST_RESULTS = res
    blocks = [
        np.asarray(res.results[i]["out"]).astype(np.float32) for i in range(N_CORES)
    ]
    return np.concatenate(blocks, axis=0)
